# revision 1
# baseline (speedup 1.0000x reference)
"""Trainium2 Bass kernel for spatial attention (GroupNorm + QKV + softmax attention
+ output projection + residual), distributed over 8 NeuronCores.

Sharding: core = 2*b + hp handles image b (of 4) and head pair hp (heads 2hp, 2hp+1).
Each core computes GroupNorm(x[b]), its heads' q/k/v, full spatial attention for its
two heads, and a partial output projection (out_w columns for its heads). Core pairs'
partial outputs are summed on the host (hp==0 core carries the residual + bias).
"""

import numpy as np

import concourse.bass as bass
import concourse.bacc as bacc
import concourse.tile as tile
from concourse import mybir
from concourse import bass_utils
from concourse.alu_op_type import AluOpType

B, C, H, W = 4, 256, 48, 48
HW = H * W  # 2304
NH, HD = 4, 64
G, GC = 16, 16  # 16 groups x 16 channels
EPS = 1e-5
NCORES = 8
JC = 128  # j (key spatial) chunk
NJ = HW // JC  # 18
IBLKS = [(0, 512), (512, 1024), (1024, 1536), (1536, 2048), (2048, 2304)]
HALF = HW // 2  # 1152

F32 = mybir.dt.float32
F32R = mybir.dt.float32r
BF16 = mybir.dt.bfloat16
AX = mybir.AxisListType.X
AF = mybir.ActivationFunctionType
OP = AluOpType


def _nchunks(size, step=512):
    # PSUM-bank-aligned chunks: a matmul output may not cross a 512-fp32 bank boundary
    return [(a, min(a + step, size)) for a in range(0, size, step)]


def _build(mm_dt=F32R):
    nc = bacc.Bacc("TRN2", target_bir_lowering=False, debug=False, enable_asserts=False)

    def cast(ap):
        return ap

    x_d = nc.dram_tensor("x", [C, HW], F32, kind="ExternalInput").ap()
    res_d = nc.dram_tensor("res", [C, HW], F32, kind="ExternalInput").ap()
    wq_d = nc.dram_tensor("wq", [C, 2 * HD], F32, kind="ExternalInput").ap()
    wk_d = nc.dram_tensor("wk", [C, 2 * HD], F32, kind="ExternalInput").ap()
    wv_d = nc.dram_tensor("wv", [C, 2 * HD], F32, kind="ExternalInput").ap()
    wo_d = nc.dram_tensor("wo", [2 * HD, C], F32, kind="ExternalInput").ap()
    gnp_d = nc.dram_tensor("gnp", [C, 2], F32, kind="ExternalInput").ap()
    gind_d = nc.dram_tensor("gind", [128, 32], F32, kind="ExternalInput").ap()
    gbc_d = nc.dram_tensor("gbc", [16, C], F32, kind="ExternalInput").ap()
    y_d = nc.dram_tensor("y", [C, HW], F32, kind="ExternalOutput").ap()

    with tile.TileContext(nc) as tc:
        with (
            tc.tile_pool(name="consts", bufs=1) as consts,
            tc.tile_pool(name="big", bufs=1) as big,
            tc.tile_pool(name="small", bufs=4) as small,
            tc.tile_pool(name="pt", bufs=4) as ptp,
        ):
            # ---- constant / weight loads ----
            gind_sb = consts.tile([128, 32], F32, tag="gind", name="gind")
            nc.sync.dma_start(gind_sb[:], gind_d[:])
            gbc_sb = consts.tile([16, C], F32, tag="gbc", name="gbc")
            nc.sync.dma_start(gbc_sb[:], gbc_d[:])
            w_sb = {}
            for name, d in (("wq", wq_d), ("wk", wk_d), ("wv", wv_d)):
                for kc in range(2):
                    tf = consts.tile([128, 2 * HD], F32, tag=f"{name}{kc}f", name=f"{name}{kc}f")
                    nc.sync.dma_start(tf[:], d[kc * 128 : (kc + 1) * 128, :])
                    t = consts.tile([128, 2 * HD], mm_dt, tag=f"{name}{kc}", name=f"{name}{kc}")
                    nc.vector.tensor_copy(t[:], tf[:])
                    w_sb[name, kc] = t
            wof = consts.tile([128, C], F32, tag="wof", name="wof")
            nc.sync.dma_start(wof[:], wo_d[:])
            wo_sb = consts.tile([128, C], mm_dt, tag="wo", name="wo")
            nc.vector.tensor_copy(wo_sb[:], wof[:])
            gnp_sb = []
            for ct in range(2):
                t = consts.tile([128, 2], F32, tag=f"gnp{ct}", name=f"gnp{ct}")
                nc.sync.dma_start(t[:], gnp_d[ct * 128 : (ct + 1) * 128, :])
                gnp_sb.append(t)

            x_sb, xn_sb, res_sb = [], [], []
            for ct in range(2):
                t = big.tile([128, HW], F32, tag=f"x{ct}", name=f"x{ct}")
                nc.sync.dma_start(t[:], x_d[ct * 128 : (ct + 1) * 128, :])
                x_sb.append(t)
                xn_sb.append(big.tile([128, HW], mm_dt, tag=f"xn{ct}", name=f"xn{ct}"))
                r = big.tile([128, HW], F32, tag=f"res{ct}", name=f"res{ct}")
                nc.sync.dma_start(r[:], res_d[ct * 128 : (ct + 1) * 128, :])
                res_sb.append(r)

            # ---- GroupNorm ----
            # per-channel sums / sum-of-squares -> stats[:, (s0,q0,s1,q1)]
            stats = small.tile([128, 4], F32, tag="stats", name="stats")
            for ct in range(2):
                nc.vector.reduce_sum(stats[:, 2 * ct : 2 * ct + 1], x_sb[ct][:], axis=AX)
                nc.vector.scalar_tensor_tensor(
                    xn_sb[ct][:], x_sb[ct][:], 1.0, x_sb[ct][:],
                    op0=OP.mult, op1=OP.mult,
                    accum_out=stats[:, 2 * ct + 1 : 2 * ct + 2],
                )
            with tc.tile_pool(name="ps_gn", bufs=2, space=bass.MemorySpace.PSUM) as ps_gn:
                # accumulate both channel tiles' per-group (sum, sumsq) into [16, 2]
                g_ps = ps_gn.tile([16, 2], F32, tag="g", name="g")
                nc.tensor.matmul(g_ps[:], gind_sb[:, 0:16], stats[:, 0:2], start=True, stop=False)
                nc.tensor.matmul(g_ps[:], gind_sb[:, 16:32], stats[:, 2:4], start=False, stop=True)
                mall = small.tile([16, 2], F32, tag="mall", name="mall")
                nc.vector.tensor_scalar_mul(mall[:], g_ps[:], 1.0 / (GC * HW))
                msq = small.tile([16, 1], F32, tag="msq", name="msq")
                nc.vector.tensor_tensor(msq[:], mall[:, 0:1], mall[:, 0:1], op=OP.mult)
                ve = small.tile([16, 1], F32, tag="ve", name="ve")
                nc.vector.tensor_tensor(ve[:], mall[:, 1:2], msq[:], op=OP.subtract)
                ve2 = small.tile([16, 1], F32, tag="ve2", name="ve2")
                nc.vector.tensor_scalar_add(ve2[:], ve[:], EPS)
                sq = small.tile([16, 1], F32, tag="sq", name="sq")
                nc.scalar.activation(sq[:], ve2[:], AF.Sqrt)
                r0 = small.tile([16, 1], F32, tag="r0", name="r0")
                nc.vector.reciprocal(r0[:], sq[:])
                # sqrt LUT is loose; one Newton step: r = r0 * (1.5 - 0.5 * v * r0^2)
                t1 = small.tile([16, 1], F32, tag="t1", name="t1")
                nc.vector.tensor_tensor(t1[:], r0[:], r0[:], op=OP.mult)
                t2 = small.tile([16, 1], F32, tag="t2", name="t2")
                nc.vector.tensor_tensor(t2[:], ve2[:], t1[:], op=OP.mult)
                t3 = small.tile([16, 1], F32, tag="t3", name="t3")
                nc.vector.tensor_scalar(t3[:], t2[:], -0.5, 1.5, op0=OP.mult, op1=OP.add)
                # gvals [16, 2] = per-group (mean, rstd)
                gvals = small.tile([16, 2], F32, tag="gvals", name="gvals")
                nc.vector.tensor_copy(gvals[:, 0:1], mall[:, 0:1])
                nc.vector.tensor_tensor(gvals[:, 1:2], r0[:], t3[:], op=OP.mult)
                for ct in range(2):
                    cv = ps_gn.tile([128, 2], F32, tag="cv", name="cv")
                    nc.tensor.matmul(
                        cv[:], gbc_sb[:, ct * 128 : (ct + 1) * 128], gvals[:],
                        start=True, stop=True,
                    )
                    scale_t = small.tile([128, 1], F32, tag="scale", name="scale")
                    nc.vector.tensor_tensor(scale_t[:], gnp_sb[ct][:, 0:1], cv[:, 1:2], op=OP.mult)
                    tb = small.tile([128, 1], F32, tag="tb", name="tb")
                    nc.vector.tensor_tensor(tb[:], cv[:, 0:1], scale_t[:], op=OP.mult)
                    bias_t = small.tile([128, 1], F32, tag="bias", name="bias")
                    nc.vector.tensor_tensor(bias_t[:], gnp_sb[ct][:, 1:2], tb[:], op=OP.subtract)
                    nc.vector.tensor_scalar(
                        xn_sb[ct][:], x_sb[ct][:], scale_t[:], bias_t[:],
                        op0=OP.mult, op1=OP.add,
                    )

            # ---- QKV projections ----
            q_sb = big.tile([128, HW], mm_dt, tag="q", name="q")
            k_sb = big.tile([128, HW], mm_dt, tag="k", name="k")
            vt_sb = []
            for h in range(2):
                t = big.tile([128, NJ * (HD + 1)], mm_dt, tag=f"vt{h}", name=f"vt{h}")
                t3 = t[:].rearrange("p (j c) -> p j c", c=HD + 1)
                # fill the per-chunk "ones" column (denominator accumulator rows);
                # memset can't write f32r, so use (x*0 + 1) via tensor_scalar
                nc.vector.tensor_scalar(
                    t3[:, :, HD : HD + 1], x_sb[0][:, 0:NJ], 0.0, 1.0,
                    op0=OP.mult, op1=OP.add,
                )
                vt_sb.append(t)

            with tc.tile_pool(name="ps_qkv", bufs=2, space=bass.MemorySpace.PSUM) as ps_qkv:
                # v^T first: its DVE copies then overlap the q/k matmuls, and
                # q/k (which attention needs first) are ready right at the boundary
                for half in range(2):
                    vps = ps_qkv.tile([128, HALF], F32, tag="qkv", name="qkv")
                    for j9 in range(9):
                        jc = half * 9 + j9
                        for kc in range(2):
                            nc.tensor.matmul(
                                vps[:, j9 * 128 : (j9 + 1) * 128],
                                xn_sb[kc][:, jc * JC : (jc + 1) * JC],
                                w_sb["wv", kc][:],
                                start=(kc == 0), stop=(kc == 1),
                            )
                    vps3 = vps[:].rearrange("p (j c) -> p j c", c=128)
                    for h in range(2):
                        dst3 = vt_sb[h][:].rearrange("p (j c) -> p j c", c=HD + 1)
                        nc.vector.tensor_copy(
                            dst3[:, half * 9 : (half + 1) * 9, 0:HD],
                            vps3[:, :, h * HD : (h + 1) * HD],
                        )
                for dst, wname in ((q_sb, "wq"), (k_sb, "wk")):
                    for half in range(2):
                        ps = ps_qkv.tile([128, HALF], F32, tag="qkv", name="qkv")
                        for kc in range(2):
                            for n0, n1 in _nchunks(HALF):
                                nc.tensor.matmul(
                                    ps[:, n0:n1],
                                    cast(w_sb[wname, kc][:]),
                                    cast(xn_sb[kc][:, half * HALF + n0 : half * HALF + n1]),
                                    start=(kc == 0), stop=(kc == 1),
                                )
                        nc.vector.tensor_copy(dst[:, half * HALF : (half + 1) * HALF], ps[:])

            # ---- attention ----
            headout = big.tile([128, HW], mm_dt, tag="headout", name="headout")
            with tc.tile_pool(name="ps_att", bufs=1, space=bass.MemorySpace.PSUM) as ps_att:
                for i0, i1 in IBLKS:
                    blk = i1 - i0
                    # h0/h1 S^T outputs must land in DIFFERENT psum banks: concurrent
                    # row-tiled matmuls writing the same bank crash the device.
                    salign = ((blk + 511) // 512) * 512
                    u = [ps_att.tile([HD + 1, blk], F32, tag=f"u{h}", name=f"u{h}", bufs=2) for h in range(2)]
                    def emit_s(jc):
                        st = ps_att.tile([128, 2 * salign], F32, tag="s", name="s", bufs=2)
                        for h in range(2):
                            lhsT = k_sb[h * HD : (h + 1) * HD, jc * JC : (jc + 1) * JC]
                            for n0, n1 in _nchunks(blk, 512):
                                nc.tensor.matmul(
                                    st[:, h * salign + n0 : h * salign + n1],
                                    cast(lhsT),
                                    cast(q_sb[h * HD : (h + 1) * HD, i0 + n0 : i0 + n1]),
                                    start=True, stop=True,
                                )
                        pt = ptp.tile([128, 2 * blk], mm_dt, tag="pt", name="pt")
                        if blk == salign:
                            nc.scalar.activation(pt[:], st[:], AF.Exp, scale=1.0 / 16.0)
                        else:
                            for h in range(2):
                                nc.scalar.activation(
                                    pt[:, h * blk : (h + 1) * blk],
                                    st[:, h * salign : h * salign + blk],
                                    AF.Exp, scale=1.0 / 16.0,
                                )
                        return pt

                    def emit_pv(jc, pt):
                        for h in range(2):
                            lhsT = vt_sb[h][:, jc * (HD + 1) : (jc + 1) * (HD + 1)]
                            for n0, n1 in _nchunks(blk, 512):
                                nc.tensor.matmul(
                                    u[h][:, n0:n1],
                                    cast(lhsT),
                                    cast(pt[:, h * blk + n0 : h * blk + n1]),
                                    start=(jc == 0), stop=(jc == NJ - 1),
                                )

                    # software-pipeline by one stage: PE computes S(jc+1) while
                    # ACT exponentiates S(jc), so the PE stream never blocks on exp
                    prev_pt = emit_s(0)
                    for jc in range(1, NJ):
                        pt = emit_s(jc)
                        emit_pv(jc - 1, prev_pt)
                        prev_pt = pt
                    emit_pv(NJ - 1, prev_pt)
                    for h in range(2):
                        dn = small.tile([1, blk], F32, tag="dn", name="dn")
                        nc.vector.tensor_copy(dn[:], u[h][HD : HD + 1, :])
                        rcp = small.tile([1, blk], F32, tag="rcp", name="rcp")
                        scr = small.tile([1, blk], F32, tag="scr", name="scr")
                        nc.vector.reciprocal_approx_accurate(rcp[:], dn[:], scr[:])
                        rb = small.tile([HD, blk], F32, tag="rb", name="rb")
                        nc.gpsimd.partition_broadcast(rb[:], rcp[:])
                        nc.vector.tensor_tensor(
                            headout[h * HD : (h + 1) * HD, i0:i1],
                            u[h][0:HD, :], rb[:], op=OP.mult,
                        )

            # ---- output projection + residual ----
            with tc.tile_pool(name="ps_out", bufs=2, space=bass.MemorySpace.PSUM) as ps_out:
                for mt in range(2):
                    for half in range(2):
                        yp = ps_out.tile([128, HALF], F32, tag="yp", name="yp")
                        for n0, n1 in _nchunks(HALF):
                            nc.tensor.matmul(
                                yp[:, n0:n1],
                                cast(wo_sb[:, mt * 128 : (mt + 1) * 128]),
                                cast(headout[:, half * HALF + n0 : half * HALF + n1]),
                                start=True, stop=True,
                            )
                        yo = small.tile([128, HALF], F32, tag="yo", name="yo")
                        nc.vector.tensor_tensor(
                            yo[:], yp[:],
                            res_sb[mt][:, half * HALF : (half + 1) * HALF], op=OP.add,
                        )
                        nc.sync.dma_start(
                            y_d[mt * 128 : (mt + 1) * 128, half * HALF : (half + 1) * HALF],
                            yo[:],
                        )

    nc.compile()
    return nc


def _consts():
    # gind[:, 0:16]: tile-0 channel -> group one-hot; [:, 16:32]: tile-1 channel -> group
    gind = np.zeros((128, 32), np.float32)
    for c in range(128):
        gind[c, c // GC] = 1.0
        gind[c, 16 + 8 + c // GC] = 1.0
    gbc = np.zeros((16, C), np.float32)
    for c in range(C):
        gbc[c // GC, c] = 1.0
    return gind, gbc


def make_in_maps(x, gn_weight, gn_bias, qkv_w, out_w, out_b):
    x = np.asarray(x, np.float32)
    qkv_w = np.asarray(qkv_w, np.float32)
    out_w = np.asarray(out_w, np.float32)
    out_b = np.asarray(out_b, np.float32)
    gn_weight = np.asarray(gn_weight, np.float32)
    gn_bias = np.asarray(gn_bias, np.float32)
    xr = np.ascontiguousarray(x.reshape(B, C, HW))
    gind, gbc = _consts()
    gnp = np.ascontiguousarray(np.stack([gn_weight, gn_bias], axis=1))
    in_maps = []
    for core in range(NCORES):
        b, hp = divmod(core, 2)
        heads = (2 * hp, 2 * hp + 1)
        qs = np.concatenate([qkv_w[n * 192 : n * 192 + 64] for n in heads], 0)
        ks = np.concatenate([qkv_w[n * 192 + 64 : n * 192 + 128] for n in heads], 0)
        vs = np.concatenate([qkv_w[n * 192 + 128 : n * 192 + 192] for n in heads], 0)
        res = xr[b] + out_b[:, None] if hp == 0 else np.zeros_like(xr[b])
        in_maps.append({
            "x": xr[b],
            "res": np.ascontiguousarray(res, np.float32),
            "wq": np.ascontiguousarray(qs.T),
            "wk": np.ascontiguousarray(ks.T),
            "wv": np.ascontiguousarray(vs.T),
            "wo": np.ascontiguousarray(out_w[:, hp * 128 : (hp + 1) * 128].T),
            "gnp": gnp,
            "gind": gind,
            "gbc": gbc,
        })
    return in_maps


_NC_CACHE = {}


def get_nc(mm_dt=F32R):
    key = str(mm_dt)
    if key not in _NC_CACHE:
        _NC_CACHE[key] = _build(mm_dt)
    return _NC_CACHE[key]


def kernel(x, gn_weight, gn_bias, qkv_w, out_w, out_b):
    nc = get_nc(BF16)
    in_maps = make_in_maps(x, gn_weight, gn_bias, qkv_w, out_w, out_b)
    res = bass_utils.run_bass_kernel_spmd(nc, in_maps, core_ids=list(range(NCORES)))
    y = np.empty((B, C, HW), np.float32)
    for b in range(B):
        y[b] = res.results[2 * b]["y"] + res.results[2 * b + 1]["y"]
    return y.reshape(B, C, H, W)



# revision 4
# speedup vs baseline: 1.0915x; 1.0915x over previous
"""Trainium2 Bass kernel for spatial attention (GroupNorm + QKV + softmax attention
+ output projection + residual), distributed over 8 NeuronCores.

Sharding: core = 2*b + hp handles image b (of 4) and head pair hp (heads 2hp, 2hp+1).
Each core computes GroupNorm(x[b]), its heads' q/k/v, full spatial attention for its
two heads, and a partial output projection (out_w columns for its heads). Core pairs'
partial outputs, the bias, and the residual are summed on the host.

Schedule notes (what makes this fast):
- x ships as bf16 and arrives in 4 chunked DMAs on 2 HWDGE queues; weights are
  pre-packed bf16 host-side so no on-device casts are needed.
- GroupNorm stats are chunked and pipelined behind the DMAs: sum(x) on DVE
  (reduce) while sum(x^2) runs on ACT (Square+accum); rstd = exp(-0.5*ln(v+eps))
  so only the ln/exp ACT table set is ever loaded.
- The attention inner loop is exp-throughput-bound on the scalar engine; a
  tunable subset of key-chunks computes exp on the vector engine instead via the
  Schraudolph bit trick: int16(s*A + B) bit-viewed as bf16 approximates exp(s/16).
- Each query-block's normalize + output projection + y DMA are emitted INSIDE the
  next block's jc loop (norm at jc==1, projection at jc==3, PV lagged by 4) so
  every engine's in-order queue keeps streaming across block boundaries and the
  PE never idles long enough to re-throttle.
"""

import math

import numpy as np

import concourse.bass as bass
import concourse.bacc as bacc
import concourse.tile as tile
from concourse import mybir
from concourse import bass_utils

B, C, H, W = 4, 256, 48, 48
HW = H * W  # 2304
NH, HD = 4, 64
G, GC = 16, 16  # 16 groups x 16 channels
EPS = 1e-5
NCORES = 8
JC = 128  # j (key spatial) chunk
NJ = HW // JC  # 18
IBLKS = [(0, 512), (512, 1024), (1024, 1536), (1536, 2048), (2048, 2304)]
HALF = HW // 2  # 1152
PV_LAG = 4  # PV trails S/exp by this many j-chunks (room for the post-chain)

F32 = mybir.dt.float32
BF16 = mybir.dt.bfloat16
I16 = mybir.dt.int16
AX = mybir.AxisListType.X
AF = mybir.ActivationFunctionType
OP = mybir.AluOpType

# which j-chunks compute exp on the vector engine (Schraudolph) instead of ACT
DVE_JCS = frozenset()

# Schraudolph constants for bf16: i16 = round(s * SCH_A + SCH_B); bits(i16) as
# bf16 approximate exp(s/16).  SCH_A = 128*log2(e)/16; SCH_B = 128*(127-sigma).
SCH_SIGMA = 0.0573
SCH_A = 128.0 * math.log2(math.e) / 16.0
SCH_B = 128.0 * (127.0 - SCH_SIGMA)

# packed bf16 weight-column layout ("wb"):
WQ0, WK0, WV0, WO0, WCOLS = 0, 256, 512, 768, 1024
# packed fp32 const-column layout ("cg"): gn params, gind, gbc
CGN0, CGIND0, CGBC0, CGCOLS = 0, 4, 36, 292


def _nchunks(size, step=512):
    # PSUM-bank-aligned chunks: a matmul output may not cross a 512-fp32 bank boundary
    return [(a, min(a + step, size)) for a in range(0, size, step)]


def _build(dve_jcs=DVE_JCS):
    nc = bacc.Bacc("TRN2", target_bir_lowering=False, debug=False, enable_asserts=False)

    xb_d = nc.dram_tensor("xb", [128, 2 * HW], BF16, kind="ExternalInput").ap()
    cg_d = nc.dram_tensor("cg", [128, CGCOLS], F32, kind="ExternalInput").ap()
    wb_d = nc.dram_tensor("wb", [128, WCOLS], BF16, kind="ExternalInput").ap()
    y_d = nc.dram_tensor("y", [C, HW], F32, kind="ExternalOutput").ap()

    with tile.TileContext(nc) as tc:
        with (
            tc.tile_pool(name="consts", bufs=1) as consts,
            tc.tile_pool(name="big", bufs=1) as big,
            tc.tile_pool(name="small", bufs=4) as small,
            tc.tile_pool(name="pt", bufs=6) as ptp,
            tc.tile_pool(name="post", bufs=2) as post,
        ):
            # ---- input DMAs: x in 4 chunks on 2 HWDGE queues, consts on gpsimd ----
            xb = big.tile([128, 2 * HW], BF16, tag="xb", name="xb")
            for chunk in range(4):
                eng = nc.sync if chunk % 2 == 0 else nc.scalar
                eng.dma_start(
                    xb[:, chunk * HALF : (chunk + 1) * HALF],
                    xb_d[:, chunk * HALF : (chunk + 1) * HALF],
                )
            cg = consts.tile([128, CGCOLS], F32, tag="cg", name="cg")
            nc.gpsimd.dma_start(cg[:], cg_d[:])
            wb = consts.tile([128, WCOLS], BF16, tag="wb", name="wb")
            nc.gpsimd.dma_start(wb[:], wb_d[:])

            def x_ct(ct):
                return xb[:, ct * HW : (ct + 1) * HW]

            # ---- GroupNorm stats (chunked): sum(x) on DVE, sum(x^2) on ACT ----
            ssum = small.tile([128, 4], F32, tag="ssum", name="ssum")
            ssq = small.tile([128, 4], F32, tag="ssq", name="ssq")
            sqscr = big.tile([128, HALF], BF16, tag="sqscr", name="sqscr")
            for chunk in range(4):
                sl = xb[:, chunk * HALF : (chunk + 1) * HALF]
                nc.vector.reduce_sum(ssum[:, chunk : chunk + 1], sl, axis=AX)
                nc.scalar.activation(
                    sqscr[:], sl, AF.Square, accum_out=ssq[:, chunk : chunk + 1],
                )

            xn_sb = [
                big.tile([128, HW], BF16, tag=f"xn{ct}", name=f"xn{ct}")
                for ct in range(2)
            ]
            with tc.tile_pool(name="ps_gn", bufs=1, space=bass.MemorySpace.PSUM) as ps_gn:
                # per-group sum / sumsq via one-hot matmuls (gind ct0 cols map
                # channels 0-127 -> groups 0-7, ct1 cols -> groups 8-15)
                g_s = ps_gn.tile([16, 1], F32, tag="g_s", name="g_s")
                g_q = ps_gn.tile([16, 1], F32, tag="g_q", name="g_q")
                for chunk in range(4):
                    ct = chunk // 2
                    gind = cg[:, CGIND0 + ct * 16 : CGIND0 + (ct + 1) * 16]
                    nc.tensor.matmul(g_s[:], gind, ssum[:, chunk : chunk + 1],
                                     start=(chunk == 0), stop=(chunk == 3))
                    nc.tensor.matmul(g_q[:], gind, ssq[:, chunk : chunk + 1],
                                     start=(chunk == 0), stop=(chunk == 3))
                gvals = small.tile([16, 2], F32, tag="gvals", name="gvals")
                nc.vector.tensor_scalar_mul(gvals[:, 0:1], g_s[:], 1.0 / (GC * HW))
                ex2 = small.tile([16, 1], F32, tag="ex2", name="ex2")
                nc.vector.tensor_scalar_mul(ex2[:], g_q[:], 1.0 / (GC * HW))
                m2 = small.tile([16, 1], F32, tag="m2", name="m2")
                nc.vector.tensor_tensor(m2[:], gvals[:, 0:1], gvals[:, 0:1], op=OP.mult)
                ve = small.tile([16, 1], F32, tag="ve", name="ve")
                nc.vector.tensor_scalar(ve[:], m2[:], -1.0, EPS, op0=OP.mult, op1=OP.add)
                ve2 = small.tile([16, 1], F32, tag="ve2", name="ve2")
                nc.vector.tensor_tensor(ve2[:], ex2[:], ve[:], op=OP.add)
                # rstd = exp(-0.5 * ln(var+eps)) -- stays within the ln/exp table set
                lnv = small.tile([16, 1], F32, tag="lnv", name="lnv")
                nc.scalar.activation(lnv[:], ve2[:], AF.Ln)
                nc.scalar.activation(gvals[:, 1:2], lnv[:], AF.Exp, scale=-0.5)
                for ct in range(2):
                    cv = ps_gn.tile([128, 2], F32, tag=f"cv{ct}", name=f"cv{ct}")
                    nc.tensor.matmul(
                        cv[:], cg[0:16, CGBC0 + ct * 128 : CGBC0 + (ct + 1) * 128],
                        gvals[:], start=True, stop=True,
                    )
                    gnw = cg[:, CGN0 + 2 * ct : CGN0 + 2 * ct + 1]
                    gnb = cg[:, CGN0 + 2 * ct + 1 : CGN0 + 2 * ct + 2]
                    scale_t = small.tile([128, 1], F32, tag="scale", name="scale")
                    nc.vector.tensor_tensor(scale_t[:], gnw, cv[:, 1:2], op=OP.mult)
                    tb = small.tile([128, 1], F32, tag="tb", name="tb")
                    nc.vector.tensor_tensor(tb[:], cv[:, 0:1], scale_t[:], op=OP.mult)
                    bias_t = small.tile([128, 1], F32, tag="bias", name="bias")
                    nc.vector.tensor_tensor(bias_t[:], gnb, tb[:], op=OP.subtract)
                    nc.vector.tensor_scalar(
                        xn_sb[ct][:], x_ct(ct), scale_t[:], bias_t[:],
                        op0=OP.mult, op1=OP.add,
                    )

            # ---- QKV projections (v, then k, then q) ----
            q_sb = big.tile([128, HW], BF16, tag="q", name="q")
            k_sb = big.tile([128, HW], BF16, tag="k", name="k")
            vt_sb = []
            for h in range(2):
                t = big.tile([128, NJ * (HD + 1)], BF16, tag=f"vt{h}", name=f"vt{h}")
                t3 = t[:].rearrange("p (j c) -> p j c", c=HD + 1)
                # per-chunk "ones" column (softmax denominator accumulator rows)
                nc.vector.tensor_scalar(
                    t3[:, :, HD : HD + 1], xb[:, 0:NJ], 0.0, 1.0,
                    op0=OP.mult, op1=OP.add,
                )
                vt_sb.append(t)

            with tc.tile_pool(name="ps_qkv", bufs=2, space=bass.MemorySpace.PSUM) as ps_qkv:
                for half in range(2):
                    vps = ps_qkv.tile([128, HALF], F32, tag="qkv", name="qkv")
                    for j9 in range(9):
                        jc = half * 9 + j9
                        for kc in range(2):
                            nc.tensor.matmul(
                                vps[:, j9 * 128 : (j9 + 1) * 128],
                                xn_sb[kc][:, jc * JC : (jc + 1) * JC],
                                wb[:, WV0 + kc * 128 : WV0 + (kc + 1) * 128],
                                start=(kc == 0), stop=(kc == 1),
                            )
                    vps3 = vps[:].rearrange("p (j c) -> p j c", c=128)
                    for h in range(2):
                        dst3 = vt_sb[h][:].rearrange("p (j c) -> p j c", c=HD + 1)
                        nc.vector.tensor_copy(
                            dst3[:, half * 9 : (half + 1) * 9, 0:HD],
                            vps3[:, :, h * HD : (h + 1) * HD],
                        )
                # k copies drain on ACT, q copies on DVE -> both finish sooner
                for dst, w0, ceng in ((k_sb, WK0, nc.scalar), (q_sb, WQ0, None)):
                    for half in range(2):
                        ps = ps_qkv.tile([128, HALF], F32, tag="qkv", name="qkv")
                        for kc in range(2):
                            for n0, n1 in _nchunks(HALF):
                                nc.tensor.matmul(
                                    ps[:, n0:n1],
                                    wb[:, w0 + kc * 128 : w0 + (kc + 1) * 128],
                                    xn_sb[kc][:, half * HALF + n0 : half * HALF + n1],
                                    start=(kc == 0), stop=(kc == 1),
                                )
                        dslice = dst[:, half * HALF : (half + 1) * HALF]
                        if ceng is not None:
                            ceng.copy(dslice, ps[:])
                        else:
                            nc.vector.tensor_copy(dslice, ps[:])

            # ---- attention with software-pipelined post-processing ----
            with (
                tc.tile_pool(name="ps_s", bufs=2, space=bass.MemorySpace.PSUM) as ps_s,
                tc.tile_pool(name="ps_u", bufs=1, space=bass.MemorySpace.PSUM) as ps_u,
                tc.tile_pool(name="ps_y", bufs=2, space=bass.MemorySpace.PSUM) as ps_y,
            ):
                post_norm = [None]
                post_proj = [None]

                def make_post(u, i0, i1):
                    blk = i1 - i0
                    headout = post.tile([128, blk], BF16, tag="ho", name="ho")

                    def norm():
                        # headout[h] = u[h][0:HD] / u[h][HD] (bcast over hd)
                        for h in range(2):
                            dn = small.tile([1, blk], F32, tag="dn", name="dn")
                            nc.vector.tensor_copy(dn[:], u[h][HD : HD + 1, 0:blk])
                            rcp = small.tile([1, blk], F32, tag="rcp", name="rcp")
                            scr = small.tile([1, blk], F32, tag="scr", name="scr")
                            nc.vector.reciprocal_approx_accurate(rcp[:], dn[:], scr[:])
                            rb = small.tile([HD, blk], F32, tag="rb", name="rb")
                            nc.gpsimd.partition_broadcast(rb[:], rcp[:])
                            nc.vector.tensor_tensor(
                                headout[h * HD : (h + 1) * HD, :],
                                u[h][0:HD, 0:blk], rb[:], op=OP.mult,
                            )

                    def proj():
                        for mt in range(2):
                            yp = ps_y.tile([128, 512], F32, tag="yp", name="yp")
                            nc.tensor.matmul(
                                yp[:, 0:blk],
                                wb[:, WO0 + mt * 128 : WO0 + (mt + 1) * 128],
                                headout[:],
                                start=True, stop=True,
                            )
                            yo = post.tile([128, blk], F32, tag=f"yo{mt}", name=f"yo{mt}")
                            nc.vector.tensor_copy(yo[:], yp[:, 0:blk])
                            nc.sync.dma_start(
                                y_d[mt * 128 : (mt + 1) * 128, i0:i1], yo[:],
                            )

                    return norm, proj

                for i0, i1 in IBLKS:
                    blk = i1 - i0
                    salign = ((blk + 511) // 512) * 512
                    # h0/h1 S^T outputs must land in DIFFERENT psum banks: concurrent
                    # row-tiled matmuls writing the same bank crash the device.
                    u = [ps_u.tile([HD + 1, 512], F32, tag=f"u{h}", name=f"u{h}")
                         for h in range(2)]

                    def emit_s(jc):
                        st = ps_s.tile([128, 2 * salign], F32, tag="s", name="s")
                        for h in range(2):
                            lhsT = k_sb[h * HD : (h + 1) * HD, jc * JC : (jc + 1) * JC]
                            for n0, n1 in _nchunks(blk, 512):
                                nc.tensor.matmul(
                                    st[:, h * salign + n0 : h * salign + n1],
                                    lhsT,
                                    q_sb[h * HD : (h + 1) * HD, i0 + n0 : i0 + n1],
                                    start=True, stop=True,
                                )
                        return st

                    def emit_exp(jc, st):
                        # returns a bf16 [128, 2*blk] AP holding ~exp(s/16)
                        if jc in dve_jcs:
                            pi = ptp.tile([128, 2 * blk], I16, tag="pti", name="pti")
                            if blk == salign:
                                nc.vector.tensor_scalar(
                                    pi[:], st[:], SCH_A, SCH_B, op0=OP.mult, op1=OP.add,
                                )
                            else:
                                for h in range(2):
                                    nc.vector.tensor_scalar(
                                        pi[:, h * blk : (h + 1) * blk],
                                        st[:, h * salign : h * salign + blk],
                                        SCH_A, SCH_B, op0=OP.mult, op1=OP.add,
                                    )
                            return pi[:].bitcast(BF16)
                        pt = ptp.tile([128, 2 * blk], BF16, tag="pt", name="pt")
                        if blk == salign:
                            nc.scalar.activation(pt[:], st[:], AF.Exp, scale=1.0 / 16.0)
                        else:
                            for h in range(2):
                                nc.scalar.activation(
                                    pt[:, h * blk : (h + 1) * blk],
                                    st[:, h * salign : h * salign + blk],
                                    AF.Exp, scale=1.0 / 16.0,
                                )
                        return pt[:]

                    def emit_pv(jc, pt):
                        for h in range(2):
                            lhsT = vt_sb[h][:, jc * (HD + 1) : (jc + 1) * (HD + 1)]
                            for n0, n1 in _nchunks(blk, 512):
                                nc.tensor.matmul(
                                    u[h][:, n0:n1],
                                    lhsT,
                                    pt[:, h * blk + n0 : h * blk + n1],
                                    start=(jc == 0), stop=(jc == NJ - 1),
                                )

                    pts = {}
                    for jc in range(NJ):
                        pts[jc] = emit_exp(jc, emit_s(jc))
                        if jc == 1 and post_norm[0] is not None:
                            post_norm[0]()
                            post_norm[0] = None
                        if jc == 3 and post_proj[0] is not None:
                            post_proj[0]()
                            post_proj[0] = None
                        if jc >= PV_LAG:
                            emit_pv(jc - PV_LAG, pts.pop(jc - PV_LAG))
                    for jc in range(NJ - PV_LAG, NJ):
                        emit_pv(jc, pts.pop(jc))
                    post_norm[0], post_proj[0] = make_post(u, i0, i1)

                # drain the last block's post-chain
                post_norm[0]()
                post_proj[0]()

    nc.compile()
    return nc


def make_in_maps(x, gn_weight, gn_bias, qkv_w, out_w, out_b):
    import ml_dtypes

    x = np.asarray(x, np.float32)
    qkv_w = np.asarray(qkv_w, np.float32)
    out_w = np.asarray(out_w, np.float32)
    gn_weight = np.asarray(gn_weight, np.float32)
    gn_bias = np.asarray(gn_bias, np.float32)
    xr = np.ascontiguousarray(x.reshape(B, C, HW))

    gind = np.zeros((128, 32), np.float32)
    for c in range(128):
        gind[c, c // GC] = 1.0           # ct0 channels -> groups 0-7
        gind[c, 16 + 8 + c // GC] = 1.0  # ct1 channels -> groups 8-15
    gbc = np.zeros((128, 256), np.float32)
    for c in range(C):
        gbc[c // GC, c] = 1.0

    cgbuf = np.zeros((128, CGCOLS), np.float32)
    cgbuf[:, CGN0 + 0] = gn_weight[0:128]
    cgbuf[:, CGN0 + 1] = gn_bias[0:128]
    cgbuf[:, CGN0 + 2] = gn_weight[128:256]
    cgbuf[:, CGN0 + 3] = gn_bias[128:256]
    cgbuf[:, CGIND0 : CGIND0 + 32] = gind
    cgbuf[:, CGBC0 : CGBC0 + 256] = gbc
    cgbuf = np.ascontiguousarray(cgbuf)

    in_maps = []
    for core in range(NCORES):
        b, hp = divmod(core, 2)
        heads = (2 * hp, 2 * hp + 1)
        qs = np.concatenate([qkv_w[n * 192 : n * 192 + 64] for n in heads], 0)
        ks = np.concatenate([qkv_w[n * 192 + 64 : n * 192 + 128] for n in heads], 0)
        vs = np.concatenate([qkv_w[n * 192 + 128 : n * 192 + 192] for n in heads], 0)

        wb = np.zeros((128, WCOLS), np.float32)
        for w0, m in ((WQ0, qs), (WK0, ks), (WV0, vs)):
            wb[:, w0 : w0 + 128] = m[:, 0:128].T
            wb[:, w0 + 128 : w0 + 256] = m[:, 128:256].T
        wb[:, WO0 : WO0 + 256] = out_w[:, hp * 128 : (hp + 1) * 128].T

        xbbuf = np.empty((128, 2 * HW), np.float32)
        xbbuf[:, 0:HW] = xr[b][0:128]
        xbbuf[:, HW : 2 * HW] = xr[b][128:256]

        in_maps.append({
            "xb": np.ascontiguousarray(xbbuf.astype(ml_dtypes.bfloat16)),
            "cg": cgbuf,
            "wb": np.ascontiguousarray(wb.astype(ml_dtypes.bfloat16)),
        })
    return in_maps


_NC_CACHE = {}


def get_nc(dve_jcs=DVE_JCS):
    key = tuple(sorted(dve_jcs))
    if key not in _NC_CACHE:
        _NC_CACHE[key] = _build(frozenset(dve_jcs))
    return _NC_CACHE[key]


def kernel(x, gn_weight, gn_bias, qkv_w, out_w, out_b):
    nc = get_nc()
    in_maps = make_in_maps(x, gn_weight, gn_bias, qkv_w, out_w, out_b)
    res = bass_utils.run_bass_kernel_spmd(nc, in_maps, core_ids=list(range(NCORES)))
    out_b = np.asarray(out_b, np.float32)
    x = np.asarray(x, np.float32)
    xr = x.reshape(B, C, HW)
    y = np.empty((B, C, HW), np.float32)
    for b in range(B):
        y[b] = res.results[2 * b]["y"] + res.results[2 * b + 1]["y"] \
            + out_b[:, None] + xr[b]
    return y.reshape(B, C, H, W)


# revision 15
# speedup vs baseline: 1.1534x; 1.0568x over previous
"""Trainium2 Bass kernel for spatial attention (GroupNorm + QKV + softmax attention
+ output projection + residual), distributed over 8 NeuronCores.

Sharding: core = 2*b + hp handles image b (of 4) and head pair hp (heads 2hp, 2hp+1).
Each core computes GroupNorm(x[b]), its heads' q/k/v, full spatial attention for its
two heads, and a partial output projection (out_w columns for its heads). Core pairs'
partial outputs, the bias, and the residual are summed on the host.

Schedule notes (what makes this fast):
- x ships as bf16 and arrives in 4 chunked DMAs on 2 HWDGE queues; weights are
  pre-packed bf16 host-side so no on-device casts are needed.
- GroupNorm stats are chunked and pipelined behind the DMAs: sum(x) on DVE
  (reduce) while sum(x^2) runs on ACT (Square+accum); rstd = exp(-0.5*ln(v+eps))
  so only the ln/exp ACT table set is ever loaded.
- The attention inner loop is exp-throughput-bound on the scalar engine; a
  tunable subset of key-chunks computes exp on the vector engine instead via the
  Schraudolph bit trick: int16(s*A + B) bit-viewed as bf16 approximates exp(s/16).
- Each query-block's normalize + output projection + y DMA are emitted INSIDE the
  next block's jc loop (norm at jc==1, projection at jc==3, PV lagged by 4) so
  every engine's in-order queue keeps streaming across block boundaries and the
  PE never idles long enough to re-throttle.
"""

import math

import numpy as np

import concourse.bass as bass
import concourse.bacc as bacc
import concourse.tile as tile
from concourse import mybir
from concourse import bass_utils

B, C, H, W = 4, 256, 48, 48
HW = H * W  # 2304
NH, HD = 4, 64
G, GC = 16, 16  # 16 groups x 16 channels
EPS = 1e-5
NCORES = 8
JC = 128  # j (key spatial) chunk
NJ = HW // JC  # 18
IBLKS = [(0, 512), (512, 1024), (1024, 1536), (1536, 2048), (2048, 2304)]
HALF = HW // 2  # 1152
PV_LAG = 4  # PV trails S/exp by this many j-chunks (room for the post-chain)
# stats chunks aligned to both the 3-way DMA split and the channel tiles:
STAT_CHUNKS = [(0, 1536), (1536, 2304), (2304, 3072), (3072, 4608)]

F32 = mybir.dt.float32
BF16 = mybir.dt.bfloat16
I16 = mybir.dt.int16
AX = mybir.AxisListType.X
AF = mybir.ActivationFunctionType
OP = mybir.AluOpType

# which j-chunks compute exp on the vector engine (Schraudolph) instead of ACT
DVE_JCS = frozenset({5, 7, 9, 11, 13, 15, 17})

# Schraudolph constants for bf16: i16 = round(s * SCH_A + SCH_B); bits(i16) as
# bf16 approximate exp(s/16).  SCH_A = 128*log2(e)/16; SCH_B = 128*(127-sigma).
SCH_SIGMA = 0.0573
SCH_A = 128.0 * math.log2(math.e) / 16.0
SCH_B = 128.0 * (127.0 - SCH_SIGMA)

# packed bf16 weight-column layout ("wb"):
WQ0, WK0, WV0, WO0, WCOLS = 0, 256, 512, 768, 1024
# packed fp32 const-column layout ("cg"): gn params, gind, gbc
CGN0, CGIND0, CGBC0, CGCOLS = 0, 4, 36, 292


def _nchunks(size, step=512):
    # PSUM-bank-aligned chunks: a matmul output may not cross a 512-fp32 bank boundary
    return [(a, min(a + step, size)) for a in range(0, size, step)]


def _build(dve_jcs=DVE_JCS):
    nc = bacc.Bacc("TRN2", target_bir_lowering=False, debug=False, enable_asserts=False)

    xb_d = nc.dram_tensor("xb", [128, 2 * HW], BF16, kind="ExternalInput").ap()
    cg_d = nc.dram_tensor("cg", [128, CGCOLS], F32, kind="ExternalInput").ap()
    wb_d = nc.dram_tensor("wb", [128, WCOLS], BF16, kind="ExternalInput").ap()
    y_d = nc.dram_tensor("y", [C, HW], F32, kind="ExternalOutput").ap()

    with tile.TileContext(nc) as tc:
        with (
            tc.tile_pool(name="consts", bufs=1) as consts,
            tc.tile_pool(name="big", bufs=1) as big,
            tc.tile_pool(name="small", bufs=4) as small,
            tc.tile_pool(name="pt", bufs=10) as ptp,
            tc.tile_pool(name="post", bufs=2) as post,
        ):
            # ---- input DMAs: x in 3 chunks on 3 queues, consts trail on 2 of them ----
            xb = big.tile([128, 2 * HW], BF16, tag="xb", name="xb")
            cg = consts.tile([128, CGCOLS], F32, tag="cg", name="cg")
            wb = consts.tile([128, WCOLS], BF16, tag="wb", name="wb")
            for eng, a, b in ((nc.sync, 0, 1536), (nc.scalar, 1536, 3072),
                              (nc.gpsimd, 3072, 4608)):
                eng.dma_start(xb[:, a:b], xb_d[:, a:b])
            nc.scalar.dma_start(cg[:], cg_d[:])
            nc.sync.dma_start(wb[:], wb_d[:])

            def x_ct(ct):
                return xb[:, ct * HW : (ct + 1) * HW]

            # ---- GroupNorm stats (chunked): sum(x) on DVE, sum(x^2) on ACT ----
            ssum = small.tile([128, 4], F32, tag="ssum", name="ssum")
            ssq = small.tile([128, 4], F32, tag="ssq", name="ssq")
            sqscr = big.tile([128, 1536], BF16, tag="sqscr", name="sqscr")
            for chunk, (a, b) in enumerate(STAT_CHUNKS):
                sl = xb[:, a:b]
                nc.vector.reduce_sum(ssum[:, chunk : chunk + 1], sl, axis=AX)
                nc.scalar.activation(
                    sqscr[:, 0 : b - a], sl, AF.Square,
                    accum_out=ssq[:, chunk : chunk + 1],
                )

            xn_sb = [
                big.tile([128, HW], BF16, tag=f"xn{ct}", name=f"xn{ct}")
                for ct in range(2)
            ]
            with tc.tile_pool(name="ps_gn", bufs=1, space=bass.MemorySpace.PSUM) as ps_gn:
                # per-group sum / sumsq via one-hot matmuls (gind ct0 cols map
                # channels 0-127 -> groups 0-7, ct1 cols -> groups 8-15)
                g_s = ps_gn.tile([16, 1], F32, tag="g_s", name="g_s")
                g_q = ps_gn.tile([16, 1], F32, tag="g_q", name="g_q")
                for chunk, (a, b) in enumerate(STAT_CHUNKS):
                    ct = a // HW
                    gind = cg[:, CGIND0 + ct * 16 : CGIND0 + (ct + 1) * 16]
                    nc.tensor.matmul(g_s[:], gind, ssum[:, chunk : chunk + 1],
                                     start=(chunk == 0), stop=(chunk == 3))
                    nc.tensor.matmul(g_q[:], gind, ssq[:, chunk : chunk + 1],
                                     start=(chunk == 0), stop=(chunk == 3))
                gvals = small.tile([16, 2], F32, tag="gvals", name="gvals")
                nc.vector.tensor_scalar_mul(gvals[:, 0:1], g_s[:], 1.0 / (GC * HW))
                ex2 = small.tile([16, 1], F32, tag="ex2", name="ex2")
                nc.vector.tensor_scalar_mul(ex2[:], g_q[:], 1.0 / (GC * HW))
                m2 = small.tile([16, 1], F32, tag="m2", name="m2")
                nc.vector.tensor_tensor(m2[:], gvals[:, 0:1], gvals[:, 0:1], op=OP.mult)
                ve = small.tile([16, 1], F32, tag="ve", name="ve")
                nc.vector.tensor_scalar(ve[:], m2[:], -1.0, EPS, op0=OP.mult, op1=OP.add)
                ve2 = small.tile([16, 1], F32, tag="ve2", name="ve2")
                nc.vector.tensor_tensor(ve2[:], ex2[:], ve[:], op=OP.add)
                # rstd = exp(-0.5 * ln(var+eps)) -- stays within the ln/exp table set
                lnv = small.tile([16, 1], F32, tag="lnv", name="lnv")
                nc.scalar.activation(lnv[:], ve2[:], AF.Ln)
                nc.scalar.activation(gvals[:, 1:2], lnv[:], AF.Exp, scale=-0.5)
                for ct in range(2):
                    cv = ps_gn.tile([128, 2], F32, tag=f"cv{ct}", name=f"cv{ct}")
                    nc.tensor.matmul(
                        cv[:], cg[0:16, CGBC0 + ct * 128 : CGBC0 + (ct + 1) * 128],
                        gvals[:], start=True, stop=True,
                    )
                    gnw = cg[:, CGN0 + 2 * ct : CGN0 + 2 * ct + 1]
                    gnb = cg[:, CGN0 + 2 * ct + 1 : CGN0 + 2 * ct + 2]
                    scale_t = small.tile([128, 1], F32, tag="scale", name="scale")
                    nc.vector.tensor_tensor(scale_t[:], gnw, cv[:, 1:2], op=OP.mult)
                    tb = small.tile([128, 1], F32, tag="tb", name="tb")
                    nc.vector.tensor_tensor(tb[:], cv[:, 0:1], scale_t[:], op=OP.mult)
                    bias_t = small.tile([128, 1], F32, tag="bias", name="bias")
                    nc.vector.tensor_tensor(bias_t[:], gnb, tb[:], op=OP.subtract)
                    nc.vector.tensor_scalar(
                        xn_sb[ct][:], x_ct(ct), scale_t[:], bias_t[:],
                        op0=OP.mult, op1=OP.add,
                    )

            # ---- QKV projections (v, then k, then q) ----
            q_sb = big.tile([128, HW], BF16, tag="q", name="q")
            k_sb = big.tile([128, HW], BF16, tag="k", name="k")
            vt_sb = []
            for h in range(2):
                t = big.tile([128, NJ * (HD + 1)], BF16, tag=f"vt{h}", name=f"vt{h}")
                t3 = t[:].rearrange("p (j c) -> p j c", c=HD + 1)
                # per-chunk "ones" column (softmax denominator accumulator rows)
                nc.vector.tensor_scalar(
                    t3[:, :, HD : HD + 1], xb[:, 0:NJ], 0.0, 1.0,
                    op0=OP.mult, op1=OP.add,
                )
                vt_sb.append(t)

            with tc.tile_pool(name="ps_qkv", bufs=2, space=bass.MemorySpace.PSUM) as ps_qkv:
                # k and q first (they gate the first S matmul); v trails since
                # PV only needs it a few microseconds later.
                # k copies drain on ACT, q copies on DVE -> both finish sooner
                for dst, w0, ceng in ((k_sb, WK0, nc.scalar), (q_sb, WQ0, None)):
                    for half in range(2):
                        ps = ps_qkv.tile([128, HALF], F32, tag="qkv", name="qkv")
                        for kc in range(2):
                            for n0, n1 in _nchunks(HALF):
                                nc.tensor.matmul(
                                    ps[:, n0:n1],
                                    wb[:, w0 + kc * 128 : w0 + (kc + 1) * 128],
                                    xn_sb[kc][:, half * HALF + n0 : half * HALF + n1],
                                    start=(kc == 0), stop=(kc == 1),
                                )
                        dslice = dst[:, half * HALF : (half + 1) * HALF]
                        if ceng is not None:
                            ceng.copy(dslice, ps[:])
                        else:
                            nc.vector.tensor_copy(dslice, ps[:])
                for half in range(2):
                    vps = ps_qkv.tile([128, HALF], F32, tag="qkv", name="qkv")
                    for j9 in range(9):
                        jc = half * 9 + j9
                        for kc in range(2):
                            nc.tensor.matmul(
                                vps[:, j9 * 128 : (j9 + 1) * 128],
                                xn_sb[kc][:, jc * JC : (jc + 1) * JC],
                                wb[:, WV0 + kc * 128 : WV0 + (kc + 1) * 128],
                                start=(kc == 0), stop=(kc == 1),
                            )
                    vps3 = vps[:].rearrange("p (j c) -> p j c", c=128)
                    for h in range(2):
                        dst3 = vt_sb[h][:].rearrange("p (j c) -> p j c", c=HD + 1)
                        nc.vector.tensor_copy(
                            dst3[:, half * 9 : (half + 1) * 9, 0:HD],
                            vps3[:, :, h * HD : (h + 1) * HD],
                        )

            # ---- attention with software-pipelined post-processing ----
            with (
                tc.tile_pool(name="ps_s", bufs=2, space=bass.MemorySpace.PSUM) as ps_s,
                tc.tile_pool(name="ps_u", bufs=1, space=bass.MemorySpace.PSUM) as ps_u,
                tc.tile_pool(name="ps_y", bufs=2, space=bass.MemorySpace.PSUM) as ps_y,
            ):
                post_norm = [None]
                post_proj = [None]
                pending_pv = [[]]

                def make_post(u, i0, i1):
                    blk = i1 - i0
                    headout = post.tile([128, blk], BF16, tag="ho", name="ho")

                    def norm():
                        # headout[h] = u[h][0:HD] / u[h][HD] (bcast over hd);
                        # interleave the two heads' chains for engine overlap
                        dns = []
                        for h in range(2):
                            dn = small.tile([1, blk], F32, tag="dn", name="dn")
                            nc.vector.tensor_copy(dn[:], u[h][HD : HD + 1, 0:blk])
                            dns.append(dn)
                        rcps = []
                        for h in range(2):
                            rcp = small.tile([1, blk], F32, tag="rcp", name="rcp")
                            scr = small.tile([1, blk], F32, tag="scr", name="scr")
                            nc.vector.reciprocal_approx_accurate(
                                rcp[:], dns[h][:], scr[:])
                            rcps.append(rcp)
                        rbs = []
                        for h in range(2):
                            rb = small.tile([HD, blk], F32, tag="rb", name="rb")
                            nc.gpsimd.partition_broadcast(rb[:], rcps[h][:])
                            rbs.append(rb)
                        for h in range(2):
                            nc.vector.tensor_tensor(
                                headout[h * HD : (h + 1) * HD, :],
                                u[h][0:HD, 0:blk], rbs[h][:], op=OP.mult,
                            )

                    def proj():
                        for mt in range(2):
                            yp = ps_y.tile([128, 512], F32, tag="yp", name="yp")
                            nc.tensor.matmul(
                                yp[:, 0:blk],
                                wb[:, WO0 + mt * 128 : WO0 + (mt + 1) * 128],
                                headout[:],
                                start=True, stop=True,
                            )
                            yo = post.tile([128, blk], F32, tag=f"yo{mt}", name=f"yo{mt}")
                            nc.vector.tensor_copy(yo[:], yp[:, 0:blk])
                            nc.sync.dma_start(
                                y_d[mt * 128 : (mt + 1) * 128, i0:i1], yo[:],
                            )

                    return norm, proj

                for i0, i1 in IBLKS:
                    blk = i1 - i0
                    salign = ((blk + 511) // 512) * 512
                    # h0/h1 S^T outputs must land in DIFFERENT psum banks: concurrent
                    # row-tiled matmuls writing the same bank crash the device.
                    u = [ps_u.tile([HD + 1, 512], F32, tag=f"u{h}", name=f"u{h}")
                         for h in range(2)]

                    def emit_s(jc):
                        st = ps_s.tile([128, 2 * salign], F32, tag="s", name="s")
                        for h in range(2):
                            lhsT = k_sb[h * HD : (h + 1) * HD, jc * JC : (jc + 1) * JC]
                            for n0, n1 in _nchunks(blk, 512):
                                nc.tensor.matmul(
                                    st[:, h * salign + n0 : h * salign + n1],
                                    lhsT,
                                    q_sb[h * HD : (h + 1) * HD, i0 + n0 : i0 + n1],
                                    start=True, stop=True,
                                )
                        return st

                    def emit_exp(jc, st):
                        # returns a bf16 [128, 2*blk] AP holding ~exp(s/16)
                        if jc in dve_jcs:
                            # write int16(s*A+B) through a bitcast view of a bf16
                            # tile: the bits then READ as ~exp(s/16) in bf16
                            pi = ptp.tile([128, 2 * blk], BF16, tag="pti", name="pti")
                            if blk == salign:
                                nc.vector.tensor_scalar(
                                    pi[:].bitcast(I16), st[:],
                                    SCH_A, SCH_B, op0=OP.mult, op1=OP.add,
                                )
                            else:
                                for h in range(2):
                                    nc.vector.tensor_scalar(
                                        pi[:, h * blk : (h + 1) * blk].bitcast(I16),
                                        st[:, h * salign : h * salign + blk],
                                        SCH_A, SCH_B, op0=OP.mult, op1=OP.add,
                                    )
                            return pi[:]
                        pt = ptp.tile([128, 2 * blk], BF16, tag="pt", name="pt")
                        if blk == salign:
                            nc.scalar.activation(pt[:], st[:], AF.Exp, scale=1.0 / 16.0)
                        else:
                            for h in range(2):
                                nc.scalar.activation(
                                    pt[:, h * blk : (h + 1) * blk],
                                    st[:, h * salign : h * salign + blk],
                                    AF.Exp, scale=1.0 / 16.0,
                                )
                        return pt[:]

                    def emit_pv(jc, pt, u=u, blk=blk):
                        # u/blk bound at def time: pending-tail calls from the
                        # next block's loop must hit THIS block's accumulators
                        for h in range(2):
                            lhsT = vt_sb[h][:, jc * (HD + 1) : (jc + 1) * (HD + 1)]
                            for n0, n1 in _nchunks(blk, 512):
                                nc.tensor.matmul(
                                    u[h][:, n0:n1],
                                    lhsT,
                                    pt[:, h * blk + n0 : h * blk + n1],
                                    start=(jc == 0), stop=(jc == NJ - 1),
                                )

                    pts = {}
                    for jc in range(NJ):
                        pts[jc] = emit_exp(jc, emit_s(jc))
                        # drain the previous block's PV tail, two pairs per slot,
                        # so the PE enters each block streaming without a bubble
                        for _ in range(2):
                            if pending_pv[0]:
                                pending_pv[0].pop(0)()
                        if jc == 2 and post_norm[0] is not None:
                            post_norm[0]()
                            post_norm[0] = None
                        if jc == 4 and post_proj[0] is not None:
                            post_proj[0]()
                            post_proj[0] = None
                        if jc >= PV_LAG:
                            emit_pv(jc - PV_LAG, pts.pop(jc - PV_LAG))
                    pending_pv[0] = [
                        (lambda jc=jc, pv=emit_pv, pt=pts.pop(jc): pv(jc, pt))
                        for jc in range(NJ - PV_LAG, NJ)
                    ]
                    post_norm[0], post_proj[0] = make_post(u, i0, i1)

                # drain the last block's PV tail and post-chain
                for fn in pending_pv[0]:
                    fn()
                post_norm[0]()
                post_proj[0]()

    nc.compile()
    return nc


def make_in_maps(x, gn_weight, gn_bias, qkv_w, out_w, out_b):
    import ml_dtypes

    x = np.asarray(x, np.float32)
    qkv_w = np.asarray(qkv_w, np.float32)
    out_w = np.asarray(out_w, np.float32)
    gn_weight = np.asarray(gn_weight, np.float32)
    gn_bias = np.asarray(gn_bias, np.float32)
    xr = np.ascontiguousarray(x.reshape(B, C, HW))

    gind = np.zeros((128, 32), np.float32)
    for c in range(128):
        gind[c, c // GC] = 1.0           # ct0 channels -> groups 0-7
        gind[c, 16 + 8 + c // GC] = 1.0  # ct1 channels -> groups 8-15
    gbc = np.zeros((128, 256), np.float32)
    for c in range(C):
        gbc[c // GC, c] = 1.0

    cgbuf = np.zeros((128, CGCOLS), np.float32)
    cgbuf[:, CGN0 + 0] = gn_weight[0:128]
    cgbuf[:, CGN0 + 1] = gn_bias[0:128]
    cgbuf[:, CGN0 + 2] = gn_weight[128:256]
    cgbuf[:, CGN0 + 3] = gn_bias[128:256]
    cgbuf[:, CGIND0 : CGIND0 + 32] = gind
    cgbuf[:, CGBC0 : CGBC0 + 256] = gbc
    cgbuf = np.ascontiguousarray(cgbuf)

    in_maps = []
    for core in range(NCORES):
        b, hp = divmod(core, 2)
        heads = (2 * hp, 2 * hp + 1)
        qs = np.concatenate([qkv_w[n * 192 : n * 192 + 64] for n in heads], 0)
        ks = np.concatenate([qkv_w[n * 192 + 64 : n * 192 + 128] for n in heads], 0)
        vs = np.concatenate([qkv_w[n * 192 + 128 : n * 192 + 192] for n in heads], 0)

        wb = np.zeros((128, WCOLS), np.float32)
        for w0, m in ((WQ0, qs), (WK0, ks), (WV0, vs)):
            wb[:, w0 : w0 + 128] = m[:, 0:128].T
            wb[:, w0 + 128 : w0 + 256] = m[:, 128:256].T
        wb[:, WO0 : WO0 + 256] = out_w[:, hp * 128 : (hp + 1) * 128].T

        xbbuf = np.empty((128, 2 * HW), np.float32)
        xbbuf[:, 0:HW] = xr[b][0:128]
        xbbuf[:, HW : 2 * HW] = xr[b][128:256]

        in_maps.append({
            "xb": np.ascontiguousarray(xbbuf.astype(ml_dtypes.bfloat16)),
            "cg": cgbuf,
            "wb": np.ascontiguousarray(wb.astype(ml_dtypes.bfloat16)),
        })
    return in_maps


_NC_CACHE = {}


def get_nc(dve_jcs=DVE_JCS):
    key = tuple(sorted(dve_jcs))
    if key not in _NC_CACHE:
        _NC_CACHE[key] = _build(frozenset(dve_jcs))
    return _NC_CACHE[key]


def kernel(x, gn_weight, gn_bias, qkv_w, out_w, out_b):
    nc = get_nc()
    in_maps = make_in_maps(x, gn_weight, gn_bias, qkv_w, out_w, out_b)
    res = bass_utils.run_bass_kernel_spmd(nc, in_maps, core_ids=list(range(NCORES)))
    out_b = np.asarray(out_b, np.float32)
    x = np.asarray(x, np.float32)
    xr = x.reshape(B, C, HW)
    y = np.empty((B, C, HW), np.float32)
    for b in range(B):
        y[b] = res.results[2 * b]["y"] + res.results[2 * b + 1]["y"] \
            + out_b[:, None] + xr[b]
    return y.reshape(B, C, H, W)
